# revision 1
# baseline (speedup 1.0000x reference)
"""DFlash draft-model (block-sparse attention + CE loss) Trainium2 kernel.

Sharding: 8 cores = 2 batches x 4 block-quarters, with anchor-block tiles
LPT-balanced across the 4 cores of each batch so every core sees a similar
context-attention width profile. Each core computes its batch's context K/V,
attention for its 32 anchor blocks (512 draft tokens), and full-vocab CE
partials (sum-exp + target logit). Host combines 8x512 scalars.

Design notes:
- fp8 DoubleRow matmuls for all projections + lm_head (2x PE).
- Attention keeps q on partitions; k/v transposes and the p^T transpose run
  on the DMA XBAR (dma_start_transpose), not the PE.
- Flash-style softmax: exp (in-place, accum row-sum on ACT) then one 4x DVE
  tensor_scalar by 1/l; masking folded into the psum->sbuf move.
- Context width per block-group is trimmed to a 512-multiple derived from
  the actual anchor values (program cached per width signature).
- RMSNorm uses Ln+Exp (exp-family ACT table) to avoid table reloads.
"""
import sys

if '/opt/trn_rl_repo' not in sys.path:
    sys.path.insert(0, '/opt/trn_rl_repo')

import numpy as np
import ml_dtypes

import concourse.bass as bass
import concourse.tile as tile
from concourse import mybir
from concourse.bass_utils import run_bass_kernel_spmd
from concourse.vector_clock import ScopedClock

BF16 = ml_dtypes.bfloat16
FP8 = ml_dtypes.float8_e4m3
F32 = mybir.dt.float32
BF = mybir.dt.bfloat16
F8 = mybir.dt.float8e4
AF = mybir.ActivationFunctionType
ALU = mybir.AluOpType
DR = mybir.MatmulPerfMode.DoubleRow

B, S, D, H, V = 2, 2048, 1024, 8, 32000
N_ANC, BS = 128, 16
HD = D // H            # 128
NCORES = 8
NB = N_ANC // 4        # 32 blocks per core
T = NB * BS            # 512 tokens per core
NG = 4                 # block-groups (128 tokens each) per core
GAMMA, EPS = 7.0, 1e-6
MASK_ID = V - 1
NEG = -30000.0
F8S = 16.0
RS = float(1.0 / np.sqrt(np.float32(HD)))
VTAIL = V - 15 * 2048          # 1280

# ---------------------------------------------------------------------------
# Workaround: this container's walrus rejects >1 sync-wait per instruction on
# the Tile kernel-tail drain; split the waits across several SP drains.
_MAX_WAITS = 1


def _patched_drain_and_barrier(self, tick_clock, wait_clock):
    nc = self.nc
    drain_inst = nc.sync.drain()
    wait_clock.add_sem_waits(
        drain_inst.ins, ScopedClock({None: tick_clock.global_clock})
    )
    si = drain_inst.ins.sync_info
    waits = list(si.on_wait)
    if len(waits) > _MAX_WAITS:
        si.on_wait = waits[:_MAX_WAITS]
        rest = waits[_MAX_WAITS:]
        for i in range(0, len(rest), _MAX_WAITS):
            extra = nc.sync.drain()
            extra.ins.sync_info = mybir.SyncInfo(
                on_update=[], on_wait=rest[i:i + _MAX_WAITS]
            )
    nc.all_engine_barrier()
    assert self.sems is not None
    popped = nc._tile_sem_poison_stack.pop()
    assert popped is self._sem_poison
    nc.clear_and_free_semaphores(list(self.sems.allocated().values()))
    nc.all_engine_barrier()


tile.TileContext._drain_and_barrier = _patched_drain_and_barrier


def _split_waits(nc, max_waits=_MAX_WAITS):
    """Walrus here allows only one sync-wait per instruction; hoist extra
    waits onto same-engine NOPs inserted immediately before the instruction
    (same engine stream order => identical semantics)."""
    for fn in nc.m.functions:
        for bb in fn.blocks:
            out = []
            changed = False
            for inst in bb.instructions:
                si = getattr(inst, "sync_info", None)
                waits = list(si.on_wait) if si is not None and si.on_wait else []
                if len(waits) > max_waits:
                    changed = True
                    keep = waits[-max_waits:]
                    rest = waits[:-max_waits]
                    for i in range(0, len(rest), max_waits):
                        nop = mybir.InstEventSemaphore(
                            name=nc.get_next_instruction_name(),
                            ins=[], outs=[])
                        nop.engine = inst.engine
                        nop.sync_info = mybir.SyncInfo(
                            on_update=[], on_wait=rest[i:i + max_waits])
                        out.append(nop)
                    si.on_wait = keep
                out.append(inst)
            if changed:
                bb.instructions = out
# ---------------------------------------------------------------------------


def _rope6(nc, pool, src, dst, cos_t, sin_t):
    """RoPE, all-bf16 on DVE (2x mode).

    src: [128, H, 2, 64] bf16 SBUF (pre-rope, possibly pre-scaled).
    dst: [128, H, 2, 64] bf16 SBUF.
    cos_t/sin_t: [128, 64] bf16 (scale factors folded in by host).
    """
    x1, x2 = src[:, :, 0, :], src[:, :, 1, :]
    cosb = cos_t[:, None, :].to_broadcast((128, H, 64))
    sinb = sin_t[:, None, :].to_broadcast((128, H, 64))
    t1 = pool.tile([128, H, 64], BF, tag="rope_t1")
    t2 = pool.tile([128, H, 64], BF, tag="rope_t2")
    nc.vector.tensor_tensor(t1[:], x1, cosb, ALU.mult)
    nc.vector.tensor_tensor(t2[:], x2, sinb, ALU.mult)
    nc.vector.tensor_tensor(dst[:, :, 0, :], t1[:], t2[:], ALU.subtract)
    nc.vector.tensor_tensor(t1[:], x1, sinb, ALU.mult)
    nc.vector.tensor_tensor(t2[:], x2, cosb, ALU.mult)
    nc.vector.tensor_tensor(dst[:, :, 1, :], t1[:], t2[:], ALU.add)


def _build_nc(widths):
    """widths: tuple of NG ints (0..4) - ctx 512-chunks per group slot."""
    nc = bass.Bass("TRN2", target_bir_lowering=False, debug=False,
                   num_devices=NCORES)
    d = {}
    def di(name, shape, dt):
        d[name] = nc.dram_tensor(name, shape, dt, kind="ExternalInput").ap()
    di("hsA", [128, 12, 4 * 2 * 512], F8)       # [p][kc2][sc,2,512]
    di("wctxA", [128, 12, 2 * 1024], F8)        # [p][kc2][2,1024]
    di("wk8", [128, 8, D], F8)
    di("wv8", [128, 8, D], F8)
    di("wq8", [128, 8, D], F8)                  # pre-scaled by 1/sqrt(HD)
    di("wo8", [128, 8, D], F8)
    di("noiseT8", [128, 8, T], F8)
    di("cosc", [128, 16, 64], BF)               # /F8S folded
    di("sinc", [128, 16, 64], BF)
    di("cosd", [128, NG, 64], BF)               # /F8S^2 folded
    di("sind", [128, NG, 64], BF)
    di("maskg", [NG, 128, S + 128], BF)         # [ctx W*512 | draft 128 | pad]
    di("lmtg", [T, D], BF)
    di("lmA", [15, 128, 2 * 8 * 1024], F8)
    di("lmB", [128, 8 * VTAIL], F8)
    di("normw", [1, D], BF)                     # norm_weight / F8S
    se = nc.dram_tensor("se", [NG, 128], F32, kind="ExternalOutput").ap()
    tl = nc.dram_tensor("tl", [NG, 128], F32, kind="ExternalOutput").ap()

    with tile.TileContext(nc) as tc:
        _body(nc, tc, d, se, tl, widths)
    _split_waits(nc)
    return nc


def _body(nc, tc, d, se_out, tl_out, widths):
    from contextlib import ExitStack
    ctx = ExitStack()
    with ctx:
        pmain = ctx.enter_context(tc.tile_pool(name="pmain", bufs=1))
        psmall = ctx.enter_context(tc.tile_pool(name="psmall", bufs=4))

        normw = pmain.tile([128, D], BF)
        nc.sync.dma_start(normw[:], d["normw"].to_broadcast((128, D)))
        eps_t = pmain.tile([128, 1], F32)
        nc.vector.memset(eps_t[:], EPS)

        oT8 = pmain.tile([128, H, T], F8)         # written D, read E
        hidT8 = pmain.tile([128, NG, 8, 128], F8) # written E, read F

        with tc.tile_pool(name="pkv", bufs=1) as pkv:
            kT = pkv.tile([128, 16, H, 128], BF)   # [hd][stile][h][s]
            vv8 = pkv.tile([128, 16, D], F8)       # [s-in-tile][stile][h*hd]

            # ---------------- Stage A: ctxT = W_ctx^T @ hs_cat^T ----------
            with tc.tile_pool(name="pA", bufs=1) as pA:
                ctxT8 = pA.tile([128, 8, S], F8)
                with tc.tile_pool(name="pAin", bufs=1) as pAin, \
                     tc.tile_pool(name="psA", bufs=2, space="PSUM") as psA:
                    hs_sb = pAin.tile([128, 12, 4, 2, 512], F8)
                    nc.sync.dma_start(
                        hs_sb[:], d["hsA"].rearrange(
                            "p k (a b c) -> p k a b c", a=4, b=2))
                    wc_sb = pAin.tile([128, 12, 2, 1024], F8)
                    nc.sync.dma_start(
                        wc_sb[:], d["wctxA"].rearrange(
                            "p k (a b) -> p k a b", a=2))
                    for sc in range(4):
                        for dh in range(2):
                            pa = psA.tile([128, 4, 512], F32, tag="actx")
                            for kc2 in range(12):
                                for dt in range(4):
                                    dta = dh * 4 + dt
                                    nc.tensor.matmul(
                                        pa[:, dt],
                                        lhsT=wc_sb[:, kc2, :,
                                                   dta * 128:(dta + 1) * 128],
                                        rhs=hs_sb[:, kc2, sc],
                                        start=(kc2 == 0), stop=(kc2 == 11),
                                        perf_mode=DR)
                            for dt in range(4):
                                dta = dh * 4 + dt
                                if dt % 2 == 0:
                                    nc.vector.tensor_scalar_mul(
                                        ctxT8[:, dta,
                                              sc * 512:(sc + 1) * 512],
                                        pa[:, dt], 1.0 / F8S)
                                else:
                                    nc.scalar.mul(
                                        ctxT8[:, dta,
                                              sc * 512:(sc + 1) * 512],
                                        pa[:, dt], 1.0 / F8S)

                # ------------ Stage B: k_ctx (rope) / v_ctx ---------------
                with tc.tile_pool(name="pB", bufs=1) as pB, \
                     tc.tile_pool(name="prope", bufs=3) as prope, \
                     tc.tile_pool(name="psB", bufs=2, space="PSUM") as psB:
                    wk_sb = pB.tile([128, 8, D], F8)
                    nc.sync.dma_start(wk_sb[:], d["wk8"][:])
                    wv_sb = pB.tile([128, 8, D], F8)
                    nc.sync.dma_start(wv_sb[:], d["wv8"][:])
                    cosc_sb = pB.tile([128, 16, 64], BF)
                    nc.sync.dma_start(cosc_sb[:], d["cosc"][:])
                    sinc_sb = pB.tile([128, 16, 64], BF)
                    nc.sync.dma_start(sinc_sb[:], d["sinc"][:])
                    for st in range(16):
                        pk = psB.tile([128, D], F32, tag="bk")
                        for half in range(2):
                            for kp in range(4):
                                nc.tensor.matmul(
                                    pk[:, half * 512:(half + 1) * 512],
                                    lhsT=ctxT8[:, 2 * kp:2 * kp + 2,
                                               st * 128:(st + 1) * 128],
                                    rhs=wk_sb[:, 2 * kp:2 * kp + 2,
                                              half * 512:(half + 1) * 512],
                                    start=(kp == 0), stop=(kp == 3),
                                    perf_mode=DR)
                        kb = prope.tile([128, H, 2, 64], BF, tag="kb")
                        nc.scalar.copy(
                            kb[:].rearrange("p a b c -> p (a b c)"), pk[:])
                        krot = prope.tile([128, H, 2, 64], BF, tag="krot")
                        _rope6(nc, prope, kb, krot,
                               cosc_sb[:, st, :], sinc_sb[:, st, :])
                        nc.sync.dma_start_transpose(
                            kT[:, st],
                            krot[:].rearrange("p a b c -> p (a b c)"))
                        pv = psB.tile([128, D], F32, tag="bv")
                        for half in range(2):
                            for kp in range(4):
                                nc.tensor.matmul(
                                    pv[:, half * 512:(half + 1) * 512],
                                    lhsT=ctxT8[:, 2 * kp:2 * kp + 2,
                                               st * 128:(st + 1) * 128],
                                    rhs=wv_sb[:, 2 * kp:2 * kp + 2,
                                              half * 512:(half + 1) * 512],
                                    start=(kp == 0), stop=(kp == 3),
                                    perf_mode=DR)
                        # vv8 keeps the 16x scale (fp8 precision); o copy
                        # divides by 16.
                        if st % 2 == 0:
                            nc.vector.tensor_scalar_mul(
                                vv8[:, st, :], pv[:], 1.0)
                        else:
                            nc.scalar.copy(vv8[:, st, :], pv[:])

            # ---------------- Stage C: draft q / k / v --------------------
            with tc.tile_pool(name="pdraft", bufs=1) as pdraft:
                qT = pdraft.tile([128, NG, H, 128], BF)
                kdT = pdraft.tile([128, NG, H, 128], BF)
                vd8 = pdraft.tile([128, NG, D], F8)

                with tc.tile_pool(name="pC", bufs=1) as pC, \
                     tc.tile_pool(name="prope2", bufs=3) as prope2, \
                     tc.tile_pool(name="psC", bufs=2, space="PSUM") as psC:
                    wq_sb = pC.tile([128, 8, D], F8)
                    nc.sync.dma_start(wq_sb[:], d["wq8"][:])
                    wk2_sb = pC.tile([128, 8, D], F8)
                    nc.sync.dma_start(wk2_sb[:], d["wk8"][:])
                    wv2_sb = pC.tile([128, 8, D], F8)
                    nc.sync.dma_start(wv2_sb[:], d["wv8"][:])
                    noise_sb = pC.tile([128, 8, T], F8)
                    nc.sync.dma_start(noise_sb[:], d["noiseT8"][:])
                    cosd_sb = pC.tile([128, NG, 64], BF)
                    nc.sync.dma_start(cosd_sb[:], d["cosd"][:])
                    sind_sb = pC.tile([128, NG, 64], BF)
                    nc.sync.dma_start(sind_sb[:], d["sind"][:])
                    for tt in range(NG):
                        for wsb, dstT in ((wq_sb, qT), (wk2_sb, kdT)):
                            pq = psC.tile([128, D], F32, tag="cq")
                            for half in range(2):
                                for kp in range(4):
                                    nc.tensor.matmul(
                                        pq[:, half * 512:(half + 1) * 512],
                                        lhsT=noise_sb[:, 2 * kp:2 * kp + 2,
                                                      tt * 128:(tt + 1) * 128],
                                        rhs=wsb[:, 2 * kp:2 * kp + 2,
                                                half * 512:(half + 1) * 512],
                                        start=(kp == 0), stop=(kp == 3),
                                        perf_mode=DR)
                            qb = prope2.tile([128, H, 2, 64], BF, tag="qb")
                            nc.scalar.copy(
                                qb[:].rearrange("p a b c -> p (a b c)"), pq[:])
                            qrot = prope2.tile([128, H, 2, 64], BF, tag="qrot")
                            _rope6(nc, prope2, qb, qrot,
                                   cosd_sb[:, tt, :], sind_sb[:, tt, :])
                            nc.sync.dma_start_transpose(
                                dstT[:, tt],
                                qrot[:].rearrange("p a b c -> p (a b c)"))
                        pq = psC.tile([128, D], F32, tag="cv")
                        for half in range(2):
                            for kp in range(4):
                                nc.tensor.matmul(
                                    pq[:, half * 512:(half + 1) * 512],
                                    lhsT=noise_sb[:, 2 * kp:2 * kp + 2,
                                                  tt * 128:(tt + 1) * 128],
                                    rhs=wv2_sb[:, 2 * kp:2 * kp + 2,
                                               half * 512:(half + 1) * 512],
                                    start=(kp == 0), stop=(kp == 3),
                                    perf_mode=DR)
                        # vd8 = 16 * v_draft (psum is 256x)
                        nc.scalar.mul(vd8[:, tt, :], pq[:], 1.0 / F8S)

                # ------------ Stage D: block-sparse attention -------------
                with tc.tile_pool(name="pattn", bufs=4) as pattn, \
                     tc.tile_pool(name="pTp", bufs=9) as pTp, \
                     tc.tile_pool(name="pmask", bufs=2) as pmask, \
                     tc.tile_pool(name="psDs", bufs=5, space="PSUM") as psDs, \
                     tc.tile_pool(name="psDd", bufs=1, space="PSUM") as psDd, \
                     tc.tile_pool(name="psDo", bufs=2, space="PSUM") as psDo:
                    for g in range(NG):
                        wg = widths[g]
                        wc = wg * 512            # ctx cols
                        wt = wc + 128            # total cols
                        nk = wg * 4              # 128-wide ctx k-tiles
                        mask_t = pmask.tile([128, S + 128], BF, tag="mask")
                        nc.sync.dma_start(mask_t[:, :wt],
                                          d["maskg"][g, :, :wt])
                        # pass 1: scores + softmax + transpose for all heads
                        pTs = []
                        for h in range(H):
                            s_sb = pattn.tile([128, 17, 128], BF, tag="s_sb")
                            sflat = s_sb[:].rearrange("p a b -> p (a b)")
                            for c in range(wg):
                                ps = psDs.tile([128, 512], F32, tag="sc")
                                nc.tensor.matmul(
                                    ps[:],
                                    lhsT=qT[:, g, h, :],
                                    rhs=kT[:, 4 * c:4 * c + 4, h, :],
                                    start=True, stop=True)
                                nc.vector.tensor_tensor(
                                    sflat[:, c * 512:(c + 1) * 512],
                                    ps[:],
                                    mask_t[:, c * 512:(c + 1) * 512],
                                    ALU.add)
                            psd = psDd.tile([128, 128], F32, tag="sd")
                            nc.tensor.matmul(
                                psd[:],
                                lhsT=qT[:, g, h, :],
                                rhs=kdT[:, g, h, :],
                                start=True, stop=True)
                            nc.vector.tensor_tensor(
                                sflat[:, wc:wt], psd[:],
                                mask_t[:, wc:wt], ALU.add)
                            l_t = psmall.tile([128, 1], F32, tag="lsum")
                            nc.scalar.activation(
                                out=sflat[:, :wt], in_=sflat[:, :wt],
                                func=AF.Exp, accum_out=l_t[:])
                            rl = psmall.tile([128, 1], F32, tag="rl")
                            nc.vector.reciprocal(rl[:], l_t[:])
                            nc.vector.tensor_scalar_mul(
                                sflat[:, :wt], sflat[:, :wt], rl[:])
                            pT = pTp.tile([128, 17, 128], BF, tag="pT")
                            nc.sync.dma_start_transpose(
                                pT[:, :nk + 1, :], sflat[:, :wt])
                            pTs.append(pT)
                        # pass 2: p @ v for all heads (PE no longer waits on
                        # the per-head softmax/transpose roundtrip)
                        for h in range(H):
                            pT = pTs[h]
                            po = psDo.tile([128, 128], F32, tag="po")
                            for kt in range(nk):
                                nc.tensor.matmul(
                                    po[:],
                                    lhsT=vv8[:, kt, h * 128:(h + 1) * 128],
                                    rhs=pT[:, kt, :],
                                    start=(kt == 0), stop=False)
                            nc.tensor.matmul(
                                po[:],
                                lhsT=vd8[:, g, h * 128:(h + 1) * 128],
                                rhs=pT[:, nk, :],
                                start=(nk == 0), stop=True)
                            # oT8 = o (psum carries 16x from v)
                            nc.scalar.mul(
                                oT8[:, h, g * 128:(g + 1) * 128], po[:],
                                1.0 / F8S)

        # ---------------- Stage E: Wo + RMSNorm + target logit ------------
        with tc.tile_pool(name="pE", bufs=2) as pE, \
             tc.tile_pool(name="pwE", bufs=1) as pwE, \
             tc.tile_pool(name="psE", bufs=2, space="PSUM") as psE:
            wo_sb = pwE.tile([128, 8, D], F8)
            nc.sync.dma_start(wo_sb[:], d["wo8"][:])
            lmtg_sb = pwE.tile([128, NG, D], BF)
            nc.sync.dma_start(
                lmtg_sb[:], d["lmtg"].rearrange("(a p) b -> p a b", a=NG))
            for tt in range(NG):
                ph = psE.tile([128, D], F32, tag="hid")
                for half in range(2):
                    for kp in range(4):
                        nc.tensor.matmul(
                            ph[:, half * 512:(half + 1) * 512],
                            lhsT=oT8[:, 2 * kp:2 * kp + 2,
                                     tt * 128:(tt + 1) * 128],
                            rhs=wo_sb[:, 2 * kp:2 * kp + 2,
                                      half * 512:(half + 1) * 512],
                            start=(kp == 0), stop=(kp == 3),
                            perf_mode=DR)
                sq = pE.tile([128, D], BF, tag="sq")
                ssq = psmall.tile([128, 1], F32, tag="ssq")
                nc.scalar.activation(out=sq[:], in_=ph[:], func=AF.Square,
                                     accum_out=ssq[:])
                # rsqrt(mean+eps) = exp(-0.5*ln(ssq/(D*256) + eps));
                # psum is 16x hid so ssq is 256x.
                lnt = psmall.tile([128, 1], F32, tag="lnt")
                nc.scalar.activation(out=lnt[:], in_=ssq[:], func=AF.Ln,
                                     bias=eps_t[:], scale=1.0 / (D * 256.0))
                rinv = psmall.tile([128, 1], F32, tag="rinv")
                nc.scalar.activation(out=rinv[:], in_=lnt[:], func=AF.Exp,
                                     scale=-0.5)
                hid_s = pE.tile([128, D], BF, tag="hids")
                nc.scalar.activation(out=hid_s[:], in_=ph[:], func=AF.Copy,
                                     scale=rinv[:])
                hid_b = pE.tile([128, D], BF, tag="hidb")
                nc.vector.tensor_tensor(hid_b[:], hid_s[:],
                                        normw[:], ALU.mult)
                tl_t = psmall.tile([128, 1], F32, tag="tlt")
                prod = pE.tile([128, D], BF, tag="tprod")
                nc.vector.tensor_tensor(prod[:], hid_b[:],
                                        lmtg_sb[:, tt, :], ALU.mult)
                nc.vector.reduce_sum(tl_t[:], prod[:],
                                     axis=mybir.AxisListType.X)
                nc.sync.dma_start(tl_out[tt, :], tl_t[:, 0])
                hidTb = pE.tile([128, 8, 128], BF, tag="hidTb")
                nc.sync.dma_start_transpose(hidTb[:], hid_b[:])
                nc.scalar.copy(
                    hidT8[:, tt].rearrange("p a b -> p (a b)"),
                    hidTb[:].rearrange("p a b -> p (a b)"))

        # ---------------- Stage F: lm_head + sum-exp ----------------------
        with tc.tile_pool(name="pF", bufs=3) as pF, \
             tc.tile_pool(name="pFs", bufs=2) as pFs, \
             tc.tile_pool(name="pFse", bufs=1) as pFse, \
             tc.tile_pool(name="psF", bufs=2, space="PSUM") as psF:
            sech = [pFse.tile([128, 16], F32, tag=f"sech{tt}",
                              name=f"sech{tt}")
                    for tt in range(NG)]
            for vp in range(16):
                lmw = pF.tile([128, 2 * 8 * 1024], F8, tag="lmw")
                if vp < 15:
                    widths_f = (1024, 1024)
                    nc.sync.dma_start(lmw[:], d["lmA"][vp])
                    lmwv = lmw[:].rearrange("p (a b c) -> p a b c", a=2, b=8)
                else:
                    widths_f = (VTAIL,)
                    nc.sync.dma_start(lmw[:, :8 * VTAIL], d["lmB"][:])
                    lmwv = lmw[:, :8 * VTAIL].rearrange(
                        "p (b c) -> p b c", b=8)[:, None]
                tot = sum(widths_f)
                for tt in range(NG):
                    ps = psF.tile([128, 2048], F32, tag="lg")
                    for vc2, vw in enumerate(widths_f):
                        for hf in range(0, vw, 512):
                            hw_ = min(512, vw - hf)
                            for k2 in range(4):
                                nc.tensor.matmul(
                                    ps[:, vc2 * 1024 + hf:
                                       vc2 * 1024 + hf + hw_],
                                    lhsT=hidT8[:, tt, 2 * k2:2 * k2 + 2, :],
                                    rhs=lmwv[:, vc2, 2 * k2:2 * k2 + 2,
                                             hf:hf + hw_],
                                    start=(k2 == 0), stop=(k2 == 3),
                                    perf_mode=DR)
                    escr = pFs.tile([128, 2048], BF, tag="escr")
                    nc.scalar.activation(
                        out=escr[:, :tot], in_=ps[:, :tot],
                        func=AF.Exp, scale=1.0 / F8S,
                        accum_out=sech[tt][:, vp:vp + 1])
            for tt in range(NG):
                se_t = psmall.tile([128, 1], F32, tag="set")
                nc.vector.reduce_sum(se_t[:], sech[tt][:],
                                     axis=mybir.AxisListType.X)
                nc.sync.dma_start(se_out[tt, :], se_t[:, 0])


_NC_CACHE = {}
_LAST_WIDTHS = None


def _get_nc(widths=None):
    global _LAST_WIDTHS
    if widths is None:
        widths = _LAST_WIDTHS
    assert widths is not None, "call _prep_core_inputs first"
    if widths not in _NC_CACHE:
        _NC_CACHE[widths] = _build_nc(widths)
    return _NC_CACHE[widths]


def _assign_tiles(anchors, keep):
    """LPT-balance 16 tiles of 8 consecutive sorted blocks over 4 cores.

    Returns per-core lists of NG tile indices (sorted desc by width) and the
    per-slot max width over the 4 cores, plus per-tile widths."""
    tw = []
    for t in range(16):
        amax = int(anchors[8 * t:8 * t + 8].max())
        tw.append((int(np.ceil(amax / 512.0)), t))
    order = sorted(tw, key=lambda x: (-x[0], x[1]))
    loads = [0] * 4
    assign = [[] for _ in range(4)]
    for w, t in order:
        c = min(range(4), key=lambda i: (loads[i], len(assign[i])))
        if len(assign[c]) >= NG:
            c = min((i for i in range(4) if len(assign[i]) < NG),
                    key=lambda i: loads[i])
        assign[c].append((w, t))
        loads[c] += w
    for c in range(4):
        assign[c].sort(key=lambda x: (-x[0], x[1]))
    return assign


def _prep_core_inputs(inputs):
    global _LAST_WIDTHS
    ids = np.asarray(inputs["input_ids"])
    hs0 = np.asarray(inputs["hs0"], dtype=np.float32)
    hs1 = np.asarray(inputs["hs1"], dtype=np.float32)
    hs2 = np.asarray(inputs["hs2"], dtype=np.float32)
    loss_mask = np.asarray(inputs["loss_mask"], dtype=np.float32)
    lm_head = np.asarray(inputs["lm_head_weight"], dtype=np.float32)
    anchors = np.asarray(inputs["anchor_positions"]).astype(np.int64)
    keep = np.asarray(inputs["block_keep_mask"]).astype(bool)
    embed = np.asarray(inputs["embed"], dtype=np.float32)
    w_ctx = np.asarray(inputs["W_ctx"], dtype=np.float32)
    wq = np.asarray(inputs["Wq"], dtype=np.float32)
    wk = np.asarray(inputs["Wk"], dtype=np.float32)
    wv = np.asarray(inputs["Wv"], dtype=np.float32)
    wo = np.asarray(inputs["Wo"], dtype=np.float32)
    norm_w = np.asarray(inputs["norm_weight"], dtype=np.float32)

    inv = (1.0 / (10000.0 ** (np.arange(64, dtype=np.float32) / np.float32(64)))
           ).astype(np.float32)
    offs = np.arange(BS)
    decay = np.exp(-np.clip(offs - 1, 0, None).astype(np.float32) / GAMMA)

    # block-tile assignment (per batch), shared slot widths
    assign_b = [_assign_tiles(anchors[b], keep[b]) for b in range(B)]
    slotw = tuple(
        max(assign_b[b][c][g][0] for b in range(B) for c in range(4))
        for g in range(NG))
    _LAST_WIDTHS = slotw

    def tile_w8(w, scale):
        # [D, D] -> [p, kc, n] fp8, scaled
        return np.ascontiguousarray(
            (w * np.float32(scale)).reshape(8, 128, D).transpose(1, 0, 2)
        ).astype(FP8)

    common = {
        "wk8": tile_w8(wk, F8S),
        "wv8": tile_w8(wv, F8S),
        "wq8": tile_w8(wq, F8S / np.sqrt(np.float32(HD))),
        "wo8": tile_w8(wo, F8S),
        "normw": (norm_w / np.float32(F8S)).reshape(1, D).astype(BF16),
    }
    # ctx position tables: cos/16 packed [p, st, 64]
    ang_c = np.arange(S, dtype=np.float32)[:, None] * inv[None, :]
    common["cosc"] = np.ascontiguousarray(
        (np.cos(ang_c) / F8S).astype(BF16).reshape(16, 128, 64)
        .transpose(1, 0, 2))
    common["sinc"] = np.ascontiguousarray(
        (np.sin(ang_c) / F8S).astype(BF16).reshape(16, 128, 64)
        .transpose(1, 0, 2))
    # wctx [12, 128, 2, 1024] * 16
    wctx8 = np.ascontiguousarray(
        (w_ctx * np.float32(F8S)).astype(FP8)
        .reshape(12, 2, 128, D).transpose(2, 0, 1, 3)).reshape(128, 12, 2 * D)
    common["wctxA"] = wctx8
    # lm head fp8 x16: [1024, 32000] -> lmA [15,128,2*8*1024], lmB
    lm8 = (np.ascontiguousarray(lm_head.T) * np.float32(F8S)).astype(FP8)
    lm8t = np.ascontiguousarray(
        lm8.reshape(8, 128, V).transpose(1, 0, 2))          # [128, 8, V]
    lmA = np.ascontiguousarray(
        lm8t[:, :, :15 * 2048].reshape(128, 8, 15, 2, 1024)
        .transpose(2, 0, 3, 1, 4)).reshape(15, 128, 2 * 8 * 1024)
    lmB = np.ascontiguousarray(
        lm8t[:, :, 15 * 2048:]).reshape(128, 8 * VTAIL)
    common["lmA"] = lmA
    common["lmB"] = lmB

    def prep_hs(b):
        hsT = np.concatenate([hs0[b], hs1[b], hs2[b]], axis=-1).T  # [3D, S]
        # [p, kc2, sc, j, s] with row r = kc2*256 + j*128 + p
        return np.ascontiguousarray(
            hsT.astype(FP8).reshape(12, 2, 128, 4, 512)
            .transpose(2, 0, 3, 1, 4)).reshape(128, 12, 4096)

    hs_by_batch = [prep_hs(b) for b in range(B)]
    e_mask = embed[MASK_ID]

    in_maps, host_w = [], []
    for c in range(NCORES):
        b, q4 = divmod(c, 4)
        tiles = assign_b[b][q4]                   # [(w, t)] x NG
        blocks = np.concatenate(
            [np.arange(8 * t, 8 * t + 8) for _, t in tiles])   # [32]
        anc = anchors[b][blocks]
        kp = keep[b][blocks]
        safe_anc = np.clip(anc, 0, S - 1)
        start_tok = np.where(kp, ids[b, safe_anc], MASK_ID)

        noise = np.broadcast_to(e_mask, (NB, BS, D)).copy()
        noise[:, 0, :] = embed[start_tok]
        noiseT8 = np.ascontiguousarray(
            (noise.reshape(T, D).T * np.float32(F8S))
            .reshape(8, 128, T).transpose(1, 0, 2)).astype(FP8)

        pos = (anc[:, None] + offs[None, :]).reshape(T)     # [512]
        ang_d = pos.astype(np.float32)[:, None] * inv[None, :]
        cosd = np.ascontiguousarray(
            (np.cos(ang_d) / (F8S * F8S)).astype(BF16)
            .reshape(NG, 128, 64).transpose(1, 0, 2))
        sind = np.ascontiguousarray(
            (np.sin(ang_d) / (F8S * F8S)).astype(BF16)
            .reshape(NG, 128, 64).transpose(1, 0, 2))

        # per-group packed mask [NG, 128, S+128]:
        # [ctx 0..wg*512 | draft 128 | pad]
        maskg = np.zeros((NG, 128, S + 128), dtype=np.float32)
        blk_eye = np.kron(np.eye(8, dtype=bool), np.ones((BS, BS), bool))
        for g in range(NG):
            wg = slotw[g]
            ga = anc[8 * g:8 * g + 8]                     # [8]
            rows_anchor = np.repeat(ga, BS)               # [128]
            cols = np.arange(wg * 512)
            mc = np.where(cols[None, :] < rows_anchor[:, None],
                          np.float32(0), np.float32(NEG))
            maskg[g, :, :wg * 512] = mc
            maskg[g, :, wg * 512:wg * 512 + 128] = np.where(
                blk_eye, np.float32(0), np.float32(NEG))
        maskg = maskg.astype(BF16)

        tpos = np.clip(pos, 0, S - 1)
        tgt = ids[b, tpos]
        lmtg = lm_head[tgt].astype(BF16)

        valid = pos < S
        j_gt0 = np.tile(offs > 0, NB)
        w = (np.repeat(kp, BS) & valid).astype(np.float32)
        w = w * j_gt0.astype(np.float32) * loss_mask[b, tpos]
        w = w * np.tile(decay, NB)
        host_w.append(w)

        im = dict(common)
        im["hsA"] = hs_by_batch[b]
        im["noiseT8"] = noiseT8
        im["cosd"] = cosd
        im["sind"] = sind
        im["maskg"] = maskg
        im["lmtg"] = lmtg
        in_maps.append(im)
    return in_maps, host_w


def _combine(results, host_w):
    num = np.float64(0.0)
    den = np.float64(0.0)
    for c in range(NCORES):
        se = np.asarray(results[c]["se"], np.float64).reshape(T)
        tl = np.asarray(results[c]["tl"], np.float64).reshape(T)
        w = host_w[c].astype(np.float64)
        lpt = np.log(np.maximum(se, 1e-300)) - tl
        num += np.sum(np.where(w > 0, lpt, 0.0) * w)
        den += np.sum(w)
    return np.float32(num / max(den, 1.0))


def kernel(**inputs):
    in_maps, host_w = _prep_core_inputs(inputs)
    nc = _get_nc()
    res = run_bass_kernel_spmd(nc, in_maps, core_ids=list(range(NCORES)))
    return _combine(res.results, host_w)

